# revision 20
# baseline (speedup 1.0000x reference)
"""MoELoRA forward kernel for 8x Trainium2 NeuronCores (Bass/Tile).

Math (see reference):
  route   = softmax(x @ W_route^T)                      [N, E]
  h       = x @ A[e,g,r,:]^T                            [N, E, G, R]
  wh      = h * route[..., None, None]
  compact = einsum(wh, Bw[e,g,o,r]) * SCALING           [N, G, OD]
  out     = zeros([N, OUT]); out[:, lora_ind] = compact.reshape(N, G*OD)

Device strategy (data-parallel over tokens, weights replicated):
  - Host pre-transposes each x shard to fp8-e3m4 xT [D, TPC]: the kernel is
    DMA-bandwidth-bound and x is the dominant input, so e3m4 (4 mantissa
    bits; empirically 1.35e-2 max rel err end-to-end vs the 2e-2 budget)
    halves the x read traffic. The contraction dim d lands on SBUF
    partitions with contiguous 512B DMA lines.
  - A is reordered to feature-major f = (g, e, r) and concatenated with
    W_route^T into one fp16 [128, KD, FE] rhs, pre-arranged on the host so
    each partition's DMA line is fully contiguous (2176B) for full-rate DMA.
    One accumulated matmul chain per 128-token tile produces h (cols
    0..127) and the routing logits (cols 128..135); fp8 lhsT x fp16 rhs is
    a legal mixed-precision matmul.
  - Softmax: exp (no max-subtract; logits are O(1)) with the row-sum fused
    into the ACT instruction via accum_out, one reciprocal, then
    probs = expv * rsum so the final PSUM->SBUF copies are scale-free and
    can run on any engine. SCALING=2 is folded into B on the host.
  - wh = h * probs uses a step-0 broadcast access pattern; wh is
    PE-transposed once per tile and the per-group up-projection runs as
    TWO K=128 matmuls of free-size 1024 against a block-diagonal fp16
    [128, 2048] B (fewer PE-SEQ instructions; PE.SEQ is near-critical).
  - The two [128,1024] fp32->fp16 PSUM drains go to Act and Pool (DVE
    keeps the softmax/wh chain), keeping every engine under the ~1.92us
    per-tile DMA cadence.
  - compact is staged fp16 in SBUF and DMAed out fp16 (halves the dominant
    write); the host upcasts and performs the lora_ind zero-pad scatter
    during unsharding.
"""

import sys
from concurrent.futures import ThreadPoolExecutor
from contextlib import ExitStack

for _p in ("/opt/trn_rl_repo", "/root/.axon_site/_ro/trn_rl_repo"):
    if _p not in sys.path:
        sys.path.insert(0, _p)

import ml_dtypes
import numpy as np

import concourse.bass as bass  # noqa: F401
import concourse.mybir as mybir
import concourse.tile as tile
from concourse import bacc
from concourse.bass_utils import run_bass_kernel_spmd
from concourse.masks import make_identity

# Problem dims (hardcoded per spec nn_MoELoRA_28089086116115)
B, S, D = 4, 4096, 1024
OUT = 3072
R, E, G = 8, 8, 2
OD = OUT // 3                    # 1024
F = G * E * R                    # 128 lora features, f = g*64 + e*8 + r
FE = F + E                       # 136: features + routing logits
SCALING = 16.0 / 8.0
NCORES = 8
NTOK = B * S                     # 16384
TPC = NTOK // NCORES             # 2048 tokens per core
TBLK = 512                       # tokens per x DMA block
NBLK = TPC // TBLK
KD = D // 128                    # 8 contraction chunks

F8 = ml_dtypes.float8_e3m4

# Hooks for test.py (not used by the grader, which calls kernel() only).
_RUN_KWARGS: dict = {}
_LAST: dict = {}

_nc_cache = None


def _build():
    f32 = mybir.dt.float32
    f16 = mybir.dt.float16
    f8 = mybir.dt.float8e3
    Exp = mybir.ActivationFunctionType.Exp
    Copy = mybir.ActivationFunctionType.Copy
    mult = mybir.AluOpType.mult

    nc = bacc.Bacc("TRN2", target_bir_lowering=False, debug=False,
                   num_devices=NCORES)
    xT = nc.dram_tensor("xT", [D, TPC], f8, kind="ExternalInput")
    awt = nc.dram_tensor("AWT", [128, KD, FE], f16, kind="ExternalInput")
    btbd = nc.dram_tensor("BT", [G, E * R, OD], f16, kind="ExternalInput")
    out = nc.dram_tensor("out", [TPC, G * OD], f16, kind="ExternalOutput")

    with tile.TileContext(nc) as tc, ExitStack() as ctx:
        wp = ctx.enter_context(tc.tile_pool(name="wp", bufs=1))
        awt_sb = wp.tile([128, KD, FE], f16)

        bt_sb = wp.tile([128, G * OD], f16)
        nc.gpsimd.memset(bt_sb[:], 0.0)
        ident = wp.tile([128, 128], f16)
        make_identity(nc, ident)

        # all x blocks live in SBUF at once (4 x 4KB/partition, fp8)
        xp = ctx.enter_context(tc.tile_pool(name="xp", bufs=NBLK))
        sp = ctx.enter_context(tc.tile_pool(name="sp", bufs=8))
        outp = ctx.enter_context(tc.tile_pool(name="outp", bufs=5))
        ph = ctx.enter_context(tc.tile_pool(name="ph", bufs=2, space="PSUM"))
        pt = ctx.enter_context(tc.tile_pool(name="pt", bufs=2, space="PSUM"))
        pc = ctx.enter_context(tc.tile_pool(name="pc", bufs=4, space="PSUM"))

        # weights first (compute needs awt + x block 0), then all x reads
        # up-front so no read ever queues behind a compute-gated write.
        nc.sync.dma_start(awt_sb[:], awt[:])
        x_sbs = []
        for blk in range(NBLK):
            x_sb = xp.tile([128, KD, TBLK], f8, name=f"x{blk}")
            xr = xT[:, blk * TBLK:(blk + 1) * TBLK].rearrange(
                "(k p) t -> p k t", p=128)
            if blk == 0:
                # split block 0 along k so the first h-matmuls start half a
                # block earlier (each k-line stays a full-rate 512B descriptor)
                nc.sync.dma_start(x_sb[:, 0:KD // 2, :], xr[:, 0:KD // 2, :])
                nc.sync.dma_start(x_sb[:, KD // 2:, :], xr[:, KD // 2:, :])
            else:
                nc.sync.dma_start(x_sb[:], xr)
            x_sbs.append(x_sb)
            if blk == 0:
                # BT is block-diagonal: zero the tile (idle Pool engine) and
                # DMA only the two nonzero 128KB blocks.
                nc.sync.dma_start(bt_sb[0:64, 0:1024], btbd[0])
                nc.sync.dma_start(bt_sb[64:128, 1024:2048], btbd[1])

        # Iteration N drains subtile N-1 (emitted at the HEAD of each
        # engine queue so the PSUM cps slots recycle before iteration N's
        # up-proj needs them), transposes/up-projects subtile N, and runs
        # the h-matmuls + softmax for subtile N+2. The two-iteration lead
        # of the softmax keeps the in-order Act/DVE queues from ever
        # delaying the loop-carried transpose chain, so the steady-state
        # cadence is DMA-paced.
        NSUB = TPC // 128
        wh_t = [None] * NSUB       # wh tiles (SBUF fp16), stage S2 output
        whT_t = [None] * NSUB      # whT tiles (SBUF fp16), stage S3 output
        cps_t = [None] * NSUB      # cps PSUM tiles, stage S4 output
        o_sbs = [None] * (NSUB // 2)

        def drain_engine(n, j):
            # Act gets j0, j2 and every other j3 (2.5/subtile); DVE the rest
            return "act" if (j in (0, 2)) or (j == 3 and n % 2 == 0) else "dve"

        for N in range(-2, NSUB + 1):
            D0 = N - 1   # subtile being drained + written
            if 0 <= D0 < NSUB:
                half = D0 % 2
                o_sb = o_sbs[D0 // 2]
                for j in range(4):
                    dst = o_sb[:, half, j * 512:(j + 1) * 512]
                    if drain_engine(D0, j) == "act":
                        nc.scalar.activation(dst, cps_t[D0][j][:], Copy)
                    else:
                        nc.vector.tensor_copy(dst, cps_t[D0][j][:])
                # output writes: edge pairs go out per-subtile (first writes
                # start a subtile earlier, the final write is half as long);
                # steady-state pairs share one 1 MiB write.
                pair = D0 // 2
                edge = pair <= 2 or pair >= NSUB // 2 - 2
                r0 = pair * 256
                if edge:
                    nc.sync.dma_start(
                        out[r0 + half * 128:r0 + (half + 1) * 128, :],
                        o_sb[:, half, :])
                elif half == 1:
                    nc.sync.dma_start(
                        out[r0:r0 + 256, :].rearrange(
                            "(s p) o -> p s o", p=128),
                        o_sb[:])

            if 0 <= N < NSUB:
                # S3: transpose + SBUF copy of subtile N's wh lead the
                # compute section of the PE and DVE queues so the up-proj
                # operand is ready before PE reaches it.
                whT_ps = pt.tile([128, 128], f16)
                nc.tensor.transpose(whT_ps[:], wh_t[N][:], ident[:])
                whT = sp.tile([128, 128], f16, tag="whT")
                nc.vector.tensor_copy(whT[:], whT_ps[:])
                whT_t[N] = whT

            K = N + 2
            if K < NSUB:
                x_sb = x_sbs[K // 4]
                t0 = (K % 4) * 128
                # S1: h (cols 0..127) + routing logits (cols 128..135)
                hE = ph.tile([128, FE], f32)
                for k in range(KD):
                    nc.tensor.matmul(
                        hE[:],
                        lhsT=x_sb[:, k, t0:t0 + 128],
                        rhs=awt_sb[:, k, :],
                        start=(k == 0),
                        stop=(k == KD - 1),
                    )
                # S2: softmax probs = exp(logits) / sum, then wh = h * probs
                expv = sp.tile([128, E], f32, tag="expv")
                ssum = sp.tile([128, 1], f32, tag="ssum")
                # plain exp (no accum_out: the accumulator read costs Act an
                # extra 187ns and Act is the tightest engine); sum on DVE
                nc.scalar.activation(expv[:], hE[:, F:FE], Exp)
                nc.vector.reduce_sum(ssum[:], expv[:],
                                     axis=mybir.AxisListType.X)
                rsum = sp.tile([128, 1], f32, tag="rsum")
                nc.vector.reciprocal(rsum[:], ssum[:])
                probs = sp.tile([128, E], f32, tag="probs")
                nc.gpsimd.tensor_scalar_mul(probs[:], expv[:], rsum[:, 0:1])
                wh = sp.tile([128, F], f16, tag="wh")
                nc.vector.tensor_tensor(
                    out=wh.rearrange("p (g e r) -> p g e r", g=G, e=E),
                    in0=hE[:, 0:F].rearrange("p (g e r) -> p g e r", g=G, e=E),
                    in1=probs[:, None, :, None].to_broadcast([128, G, E, R]),
                    op=mult,
                )
                wh_t[K] = wh

            if 0 <= N < NSUB:
                if N % 2 == 0:
                    o_sbs[N // 2] = outp.tile([128, 2, G * OD], f16,
                                              name=f"o{N // 2}", tag="o")
                # S4: compact[t, (g,o)] via block-diagonal 2*B^T (K=128),
                # one PSUM bank per 512-col matmul; drained next iteration
                cps_l = []
                for j in range(4):
                    cps = pc.tile([128, 512], f32, name=f"cps{j}", tag="cps")
                    nc.tensor.matmul(
                        cps[:],
                        lhsT=whT_t[N][:],
                        rhs=bt_sb[:, j * 512:(j + 1) * 512],
                        start=True,
                        stop=True,
                    )
                    cps_l.append(cps)
                cps_t[N] = cps_l

    nc.compile()
    return nc


def _shard_xT(x, c):
    return np.ascontiguousarray(x[c * TPC:(c + 1) * TPC].T).astype(F8)


_runner = None


def _get_runner(nc):
    """Build the sharded PJRT callable once; reuse across kernel() calls.

    Mirrors bass2jax.run_bass_via_pjrt's multi-core branch, but caches the
    jitted function so repeat calls skip retrace/recompile. Falls back to
    the stock path (handled by caller) on any failure.
    """
    global _runner
    if _runner is not None:
        return _runner
    import jax
    from jax.experimental.shard_map import shard_map
    from jax.sharding import Mesh, PartitionSpec

    from concourse import bass2jax, mybir as _mb

    bass2jax.install_neuronx_cc_hook()
    partition_name = (nc.partition_id_tensor.name
                      if nc.partition_id_tensor else None)
    in_names, out_names, out_avals = [], [], []
    for alloc in nc.m.functions[0].allocations:
        if not isinstance(alloc, _mb.MemoryLocationSet):
            continue
        name = alloc.memorylocations[0].name
        if alloc.kind == "ExternalInput":
            if name != partition_name:
                in_names.append(name)
        elif alloc.kind == "ExternalOutput":
            out_names.append(name)
            out_avals.append(jax.core.ShapedArray(
                tuple(alloc.tensor_shape), _mb.dt.np(alloc.dtype)))
    n_params = len(in_names)
    n_outs = len(out_avals)
    all_in_names = list(in_names) + list(out_names)
    if partition_name is not None:
        all_in_names.append(partition_name)

    def _body(*args):
        operands = list(args)
        if partition_name is not None:
            operands.append(bass2jax.partition_id_tensor())
        outs = bass2jax._bass_exec_p.bind(
            *operands,
            out_avals=tuple(out_avals),
            in_names=tuple(all_in_names),
            out_names=tuple(out_names),
            lowering_input_output_aliases=(),
            sim_require_finite=True,
            sim_require_nnan=True,
            nc=nc,
        )
        return tuple(outs)

    devices = jax.devices()[:NCORES]
    mesh = Mesh(np.asarray(devices), ("core",))
    specs = (PartitionSpec("core"),) * (n_params + n_outs)
    sharded = jax.jit(
        shard_map(_body, mesh=mesh, in_specs=specs,
                  out_specs=(PartitionSpec("core"),) * n_outs,
                  check_rep=False),
        donate_argnums=tuple(range(n_params, n_params + n_outs)),
        keep_unused=True,
    )
    _runner = (sharded, in_names, out_names, out_avals)
    return _runner


def _run_cached(nc, in_maps):
    sharded, in_names, out_names, out_avals = _get_runner(nc)
    concat_in = [
        np.concatenate([np.asarray(m[name]) for m in in_maps], axis=0)
        for name in in_names
    ]
    concat_zeros = [
        np.zeros((NCORES * a.shape[0], *a.shape[1:]), a.dtype)
        for a in out_avals
    ]
    out_arrs = sharded(*concat_in, *concat_zeros)
    return [
        {name: np.asarray(out_arrs[i]).reshape(NCORES, *out_avals[i].shape)[c]
         for i, name in enumerate(out_names)}
        for c in range(NCORES)
    ]


def kernel(x, W_route, A, Bw, lora_ind):
    global _nc_cache
    x = np.asarray(x, dtype=np.float32).reshape(NTOK, D)
    W_route = np.asarray(W_route, dtype=np.float32)
    A = np.asarray(A, dtype=np.float32)
    Bw = np.asarray(Bw, dtype=np.float32)
    lora_ind = np.asarray(lora_ind).astype(np.int64)

    # [D, 136] fp16: cols 0..127 are A rows in (g, e, r) order, 128.. W_route;
    # re-packed to [128, KD, FE] so each partition's DMA line is contiguous.
    A_all = A.transpose(1, 0, 2, 3).reshape(F, D)
    AWT = np.concatenate([A_all.T, W_route.T], axis=1).astype(np.float16)
    AWT_dev = np.ascontiguousarray(
        AWT.reshape(KD, 128, FE).transpose(1, 0, 2))
    # block-diagonal B^T with SCALING folded in: rows (g,e,r), cols (g,o)
    BTbd = (Bw.transpose(1, 0, 3, 2).reshape(G, E * R, OD)
            * SCALING).astype(np.float16)

    if _nc_cache is None:
        _nc_cache = _build()
    nc = _nc_cache

    with ThreadPoolExecutor(NCORES) as ex:
        xTs = list(ex.map(lambda c: _shard_xT(x, c), range(NCORES)))
    in_maps = [{"xT": xTs[c], "AWT": AWT_dev, "BT": BTbd}
               for c in range(NCORES)]

    try:
        results = _run_cached(nc, in_maps)
    except Exception:  # noqa: BLE001  (fall back to the stock SPMD path)
        global _runner
        _runner = None
        res = run_bass_kernel_spmd(nc, in_maps, core_ids=list(range(NCORES)),
                                   **_RUN_KWARGS)
        results = res.results
    _LAST["results"] = results

    compact = np.concatenate(
        [results[c]["out"] for c in range(NCORES)], axis=0)
    outp = np.zeros((NTOK, OUT), dtype=np.float32)
    outp[:, lora_ind] = compact.astype(np.float32)
    return outp.reshape(B, S, OUT)


# revision 26
# speedup vs baseline: 1.0300x; 1.0300x over previous
"""MoELoRA forward kernel for 8x Trainium2 NeuronCores (Bass/Tile).

Math (see reference):
  route   = softmax(x @ W_route^T)                      [N, E]
  h       = x @ A[e,g,r,:]^T                            [N, E, G, R]
  wh      = h * route[..., None, None]
  compact = einsum(wh, Bw[e,g,o,r]) * SCALING           [N, G, OD]
  out     = zeros([N, OUT]); out[:, lora_ind] = compact.reshape(N, G*OD)

Device strategy (data-parallel over tokens, weights replicated):
  - Host pre-transposes each x shard to fp8-e3m4 xT [D, TPC]: the kernel is
    DMA-bandwidth-bound and x is the dominant input, so e3m4 (4 mantissa
    bits; empirically 1.35e-2 max rel err end-to-end vs the 2e-2 budget)
    halves the x read traffic. The contraction dim d lands on SBUF
    partitions with contiguous 512B DMA lines.
  - A is reordered to feature-major f = (g, e, r) and concatenated with
    W_route^T into one fp16 [128, KD, FE] rhs, pre-arranged on the host so
    each partition's DMA line is fully contiguous (2176B) for full-rate DMA.
    One accumulated matmul chain per 128-token tile produces h (cols
    0..127) and the routing logits (cols 128..135); fp8 lhsT x fp16 rhs is
    a legal mixed-precision matmul.
  - Softmax: exp (no max-subtract; logits are O(1)) with the row-sum fused
    into the ACT instruction via accum_out, one reciprocal, then
    probs = expv * rsum so the final PSUM->SBUF copies are scale-free and
    can run on any engine. SCALING=2 is folded into B on the host.
  - wh = h * probs uses a step-0 broadcast access pattern; wh is
    PE-transposed once per tile and the per-group up-projection runs as
    TWO K=128 matmuls of free-size 1024 against a block-diagonal fp16
    [128, 2048] B (fewer PE-SEQ instructions; PE.SEQ is near-critical).
  - The two [128,1024] fp32->fp16 PSUM drains go to Act and Pool (DVE
    keeps the softmax/wh chain), keeping every engine under the ~1.92us
    per-tile DMA cadence.
  - compact is staged fp16 in SBUF and DMAed out fp16 (halves the dominant
    write); the host upcasts and performs the lora_ind zero-pad scatter
    during unsharding.
"""

import sys
from concurrent.futures import ThreadPoolExecutor
from contextlib import ExitStack

for _p in ("/opt/trn_rl_repo", "/root/.axon_site/_ro/trn_rl_repo"):
    if _p not in sys.path:
        sys.path.insert(0, _p)

import ml_dtypes
import numpy as np

import concourse.bass as bass  # noqa: F401
import concourse.mybir as mybir
import concourse.tile as tile
from concourse import bacc
from concourse.bass_utils import run_bass_kernel_spmd
from concourse.masks import make_identity

# Problem dims (hardcoded per spec nn_MoELoRA_28089086116115)
B, S, D = 4, 4096, 1024
OUT = 3072
R, E, G = 8, 8, 2
OD = OUT // 3                    # 1024
F = G * E * R                    # 128 lora features, f = g*64 + e*8 + r
FE = F + E                       # 136: features + routing logits
SCALING = 16.0 / 8.0
NCORES = 8
NTOK = B * S                     # 16384
TPC = NTOK // NCORES             # 2048 tokens per core
TBLK = 512                       # tokens per x DMA block
NBLK = TPC // TBLK
KD = D // 128                    # 8 contraction chunks

F8 = ml_dtypes.float8_e3m4

# Hooks for test.py (not used by the grader, which calls kernel() only).
_RUN_KWARGS: dict = {}
_LAST: dict = {}

_nc_cache = None


def _build():
    f32 = mybir.dt.float32
    f16 = mybir.dt.float16
    f8 = mybir.dt.float8e3
    Exp = mybir.ActivationFunctionType.Exp
    Copy = mybir.ActivationFunctionType.Copy
    mult = mybir.AluOpType.mult

    nc = bacc.Bacc("TRN2", target_bir_lowering=False, debug=False,
                   num_devices=NCORES)
    xT = nc.dram_tensor("xT", [D, TPC], f8, kind="ExternalInput")
    awt = nc.dram_tensor("AWT", [128, KD, FE], f16, kind="ExternalInput")
    btbd = nc.dram_tensor("BT", [G, E * R, OD], f16, kind="ExternalInput")
    out = nc.dram_tensor("out", [TPC, G * OD], f16, kind="ExternalOutput")

    with tile.TileContext(nc) as tc, ExitStack() as ctx:
        wp = ctx.enter_context(tc.tile_pool(name="wp", bufs=1))
        awt_sb = wp.tile([128, KD, FE], f16)

        bt_sb = wp.tile([128, G * OD], f16)
        nc.gpsimd.memset(bt_sb[:], 0.0)
        ident = wp.tile([128, 128], f16)
        make_identity(nc, ident)



        # all x blocks live in SBUF at once (4 x 4KB/partition, fp8)
        xp = ctx.enter_context(tc.tile_pool(name="xp", bufs=NBLK))
        sp = ctx.enter_context(tc.tile_pool(name="sp", bufs=8))
        outp = ctx.enter_context(tc.tile_pool(name="outp", bufs=5))
        ph = ctx.enter_context(tc.tile_pool(name="ph", bufs=2, space="PSUM"))
        pt = ctx.enter_context(tc.tile_pool(name="pt", bufs=2, space="PSUM"))
        pc = ctx.enter_context(tc.tile_pool(name="pc", bufs=4, space="PSUM"))

        # Warm the PE p-state ramp during the DMA preamble: ~3us of dummy
        # back-to-back transposes (rotating through the pt slots; nothing
        # reads them) bring the tensor engine to full clock before the
        # first real h-matmul arrives — cold-start matmuls otherwise run at
        # 2-4x cycle time for the first 3us of busy.
        warm_ps = pt.tile([128, 128], f16, name="warm", tag="whT_ps")
        for _ in range(30):
            nc.tensor.transpose(warm_ps[:], ident[:], ident[:])

        # weights first (compute needs awt + x block 0), then all x reads
        # up-front so no read ever queues behind a compute-gated write.
        nc.sync.dma_start(awt_sb[:], awt[:])
        x_sbs = []
        for blk in range(NBLK):
            x_sb = xp.tile([128, KD, TBLK], f8, name=f"x{blk}")
            xr = xT[:, blk * TBLK:(blk + 1) * TBLK].rearrange(
                "(k p) t -> p k t", p=128)
            if blk == 0:
                # split block 0 along k so the first h-matmuls start half a
                # block earlier (each k-line stays a full-rate 512B descriptor)
                nc.sync.dma_start(x_sb[:, 0:KD // 2, :], xr[:, 0:KD // 2, :])
                nc.sync.dma_start(x_sb[:, KD // 2:, :], xr[:, KD // 2:, :])
            else:
                nc.sync.dma_start(x_sb[:], xr)
            x_sbs.append(x_sb)
            if blk == 0:
                # BT is block-diagonal: zero the tile (idle Pool engine) and
                # DMA only the two nonzero 128KB blocks.
                nc.sync.dma_start(bt_sb[0:64, 0:1024], btbd[0])
                nc.sync.dma_start(bt_sb[64:128, 1024:2048], btbd[1])

        # Iteration N drains subtile N-1 (emitted at the HEAD of each
        # engine queue so the PSUM cps slots recycle before iteration N's
        # up-proj needs them), transposes/up-projects subtile N, and runs
        # the h-matmuls + softmax for subtile N+2. The two-iteration lead
        # of the softmax keeps the in-order Act/DVE queues from ever
        # delaying the loop-carried transpose chain, so the steady-state
        # cadence is DMA-paced.
        NSUB = TPC // 128
        wh_t = [None] * NSUB       # wh tiles (SBUF fp16), stage S2 output
        whT_t = [None] * NSUB      # whT tiles (SBUF fp16), stage S3 output
        cps_t = [None] * NSUB      # cps PSUM tiles, stage S4 output
        o_sbs = [None] * (NSUB // 2)

        def drain_engine(n, j):
            # Act gets j0, j2 and every other j3 (2.5/subtile); DVE the rest
            return "act" if (j in (0, 2)) or (j == 3 and n % 2 == 0) else "dve"

        for N in range(-2, NSUB + 1):
            D0 = N - 1   # subtile being drained + written
            if 0 <= D0 < NSUB:
                half = D0 % 2
                o_sb = o_sbs[D0 // 2]
                for j in range(4):
                    dst = o_sb[:, half, j * 512:(j + 1) * 512]
                    if drain_engine(D0, j) == "act":
                        nc.scalar.activation(dst, cps_t[D0][j][:], Copy)
                    else:
                        nc.vector.tensor_copy(dst, cps_t[D0][j][:])
                # output writes: edge pairs go out per-subtile (first writes
                # start a subtile earlier, the final write is half as long);
                # steady-state pairs share one 1 MiB write.
                pair = D0 // 2
                edge = pair <= 2 or pair >= NSUB // 2 - 2
                r0 = pair * 256
                if D0 == NSUB - 1:
                    # very last write: split by column halves so the first
                    # 0.25 MiB goes out while j2/j3 still drain, and the
                    # final transfer on the critical tail is half as long
                    rows = out[r0 + 128:r0 + 256, :]
                    nc.sync.dma_start(rows[:, 0:1024], o_sb[:, 1, 0:1024])
                    nc.sync.dma_start(rows[:, 1024:2048],
                                      o_sb[:, 1, 1024:2048])
                elif edge:
                    nc.sync.dma_start(
                        out[r0 + half * 128:r0 + (half + 1) * 128, :],
                        o_sb[:, half, :])
                elif half == 1:
                    nc.sync.dma_start(
                        out[r0:r0 + 256, :].rearrange(
                            "(s p) o -> p s o", p=128),
                        o_sb[:])

            if 0 <= N < NSUB:
                # S3: transpose + SBUF copy of subtile N's wh lead the
                # compute section of the PE and DVE queues so the up-proj
                # operand is ready before PE reaches it.
                whT_ps = pt.tile([128, 128], f16)
                nc.tensor.transpose(whT_ps[:], wh_t[N][:], ident[:])
                whT = sp.tile([128, 128], f16, tag="whT")
                nc.vector.tensor_copy(whT[:], whT_ps[:])
                whT_t[N] = whT

            K = N + 2
            if K < NSUB:
                x_sb = x_sbs[K // 4]
                t0 = (K % 4) * 128
                # S1: h (cols 0..127) + routing logits (cols 128..135)
                hE = ph.tile([128, FE], f32)
                for k in range(KD):
                    nc.tensor.matmul(
                        hE[:],
                        lhsT=x_sb[:, k, t0:t0 + 128],
                        rhs=awt_sb[:, k, :],
                        start=(k == 0),
                        stop=(k == KD - 1),
                    )
                # S2: softmax probs = exp(logits) / sum, then wh = h * probs
                expv = sp.tile([128, E], f32, tag="expv")
                ssum = sp.tile([128, 1], f32, tag="ssum")
                # plain exp (no accum_out: the accumulator read costs Act an
                # extra 187ns and Act is the tightest engine); sum on DVE
                nc.scalar.activation(expv[:], hE[:, F:FE], Exp)
                nc.vector.reduce_sum(ssum[:], expv[:],
                                     axis=mybir.AxisListType.X)
                rsum = sp.tile([128, 1], f32, tag="rsum")
                nc.vector.reciprocal(rsum[:], ssum[:])
                probs = sp.tile([128, E], f32, tag="probs")
                nc.gpsimd.tensor_scalar_mul(probs[:], expv[:], rsum[:, 0:1])
                wh = sp.tile([128, F], f16, tag="wh")
                nc.vector.tensor_tensor(
                    out=wh.rearrange("p (g e r) -> p g e r", g=G, e=E),
                    in0=hE[:, 0:F].rearrange("p (g e r) -> p g e r", g=G, e=E),
                    in1=probs[:, None, :, None].to_broadcast([128, G, E, R]),
                    op=mult,
                )
                wh_t[K] = wh

            if 0 <= N < NSUB:
                if N % 2 == 0:
                    o_sbs[N // 2] = outp.tile([128, 2, G * OD], f16,
                                              name=f"o{N // 2}", tag="o")
                # S4: compact[t, (g,o)] via block-diagonal 2*B^T (K=128),
                # one PSUM bank per 512-col matmul; drained next iteration
                cps_l = []
                for j in range(4):
                    cps = pc.tile([128, 512], f32, name=f"cps{j}", tag="cps")
                    nc.tensor.matmul(
                        cps[:],
                        lhsT=whT_t[N][:],
                        rhs=bt_sb[:, j * 512:(j + 1) * 512],
                        start=True,
                        stop=True,
                    )
                    cps_l.append(cps)
                cps_t[N] = cps_l

    nc.compile()
    return nc


def _shard_xT(x, c):
    return np.ascontiguousarray(x[c * TPC:(c + 1) * TPC].T).astype(F8)


_runner = None


def _get_runner(nc):
    """Build the sharded PJRT callable once; reuse across kernel() calls.

    Mirrors bass2jax.run_bass_via_pjrt's multi-core branch, but caches the
    jitted function so repeat calls skip retrace/recompile. Falls back to
    the stock path (handled by caller) on any failure.
    """
    global _runner
    if _runner is not None:
        return _runner
    import jax
    from jax.experimental.shard_map import shard_map
    from jax.sharding import Mesh, PartitionSpec

    from concourse import bass2jax, mybir as _mb

    bass2jax.install_neuronx_cc_hook()
    partition_name = (nc.partition_id_tensor.name
                      if nc.partition_id_tensor else None)
    in_names, out_names, out_avals = [], [], []
    for alloc in nc.m.functions[0].allocations:
        if not isinstance(alloc, _mb.MemoryLocationSet):
            continue
        name = alloc.memorylocations[0].name
        if alloc.kind == "ExternalInput":
            if name != partition_name:
                in_names.append(name)
        elif alloc.kind == "ExternalOutput":
            out_names.append(name)
            out_avals.append(jax.core.ShapedArray(
                tuple(alloc.tensor_shape), _mb.dt.np(alloc.dtype)))
    n_params = len(in_names)
    n_outs = len(out_avals)
    all_in_names = list(in_names) + list(out_names)
    if partition_name is not None:
        all_in_names.append(partition_name)

    def _body(*args):
        operands = list(args)
        if partition_name is not None:
            operands.append(bass2jax.partition_id_tensor())
        outs = bass2jax._bass_exec_p.bind(
            *operands,
            out_avals=tuple(out_avals),
            in_names=tuple(all_in_names),
            out_names=tuple(out_names),
            lowering_input_output_aliases=(),
            sim_require_finite=True,
            sim_require_nnan=True,
            nc=nc,
        )
        return tuple(outs)

    devices = jax.devices()[:NCORES]
    mesh = Mesh(np.asarray(devices), ("core",))
    specs = (PartitionSpec("core"),) * (n_params + n_outs)
    sharded = jax.jit(
        shard_map(_body, mesh=mesh, in_specs=specs,
                  out_specs=(PartitionSpec("core"),) * n_outs,
                  check_rep=False),
        donate_argnums=tuple(range(n_params, n_params + n_outs)),
        keep_unused=True,
    )
    _runner = (sharded, in_names, out_names, out_avals)
    return _runner


def _run_cached(nc, in_maps):
    sharded, in_names, out_names, out_avals = _get_runner(nc)
    concat_in = [
        np.concatenate([np.asarray(m[name]) for m in in_maps], axis=0)
        for name in in_names
    ]
    concat_zeros = [
        np.zeros((NCORES * a.shape[0], *a.shape[1:]), a.dtype)
        for a in out_avals
    ]
    out_arrs = sharded(*concat_in, *concat_zeros)
    return [
        {name: np.asarray(out_arrs[i]).reshape(NCORES, *out_avals[i].shape)[c]
         for i, name in enumerate(out_names)}
        for c in range(NCORES)
    ]


def kernel(x, W_route, A, Bw, lora_ind):
    global _nc_cache
    x = np.asarray(x, dtype=np.float32).reshape(NTOK, D)
    W_route = np.asarray(W_route, dtype=np.float32)
    A = np.asarray(A, dtype=np.float32)
    Bw = np.asarray(Bw, dtype=np.float32)
    lora_ind = np.asarray(lora_ind).astype(np.int64)

    # [D, 136] fp16: cols 0..127 are A rows in (g, e, r) order, 128.. W_route;
    # re-packed to [128, KD, FE] so each partition's DMA line is contiguous.
    A_all = A.transpose(1, 0, 2, 3).reshape(F, D)
    AWT = np.concatenate([A_all.T, W_route.T], axis=1).astype(np.float16)
    AWT_dev = np.ascontiguousarray(
        AWT.reshape(KD, 128, FE).transpose(1, 0, 2))
    # block-diagonal B^T with SCALING folded in: rows (g,e,r), cols (g,o)
    BTbd = (Bw.transpose(1, 0, 3, 2).reshape(G, E * R, OD)
            * SCALING).astype(np.float16)

    if _nc_cache is None:
        _nc_cache = _build()
    nc = _nc_cache

    with ThreadPoolExecutor(NCORES) as ex:
        xTs = list(ex.map(lambda c: _shard_xT(x, c), range(NCORES)))
    in_maps = [{"xT": xTs[c], "AWT": AWT_dev, "BT": BTbd}
               for c in range(NCORES)]

    try:
        results = _run_cached(nc, in_maps)
    except Exception:  # noqa: BLE001  (fall back to the stock SPMD path)
        global _runner
        _runner = None
        res = run_bass_kernel_spmd(nc, in_maps, core_ids=list(range(NCORES)),
                                   **_RUN_KWARGS)
        results = res.results
    _LAST["results"] = results

    compact = np.concatenate(
        [results[c]["out"] for c in range(NCORES)], axis=0)
    outp = np.zeros((NTOK, OUT), dtype=np.float32)
    outp[:, lora_ind] = compact.astype(np.float32)
    return outp.reshape(B, S, OUT)


# revision 27
# speedup vs baseline: 1.0313x; 1.0013x over previous
"""MoELoRA forward kernel for 8x Trainium2 NeuronCores (Bass/Tile).

Math (see reference):
  route   = softmax(x @ W_route^T)                      [N, E]
  h       = x @ A[e,g,r,:]^T                            [N, E, G, R]
  wh      = h * route[..., None, None]
  compact = einsum(wh, Bw[e,g,o,r]) * SCALING           [N, G, OD]
  out     = zeros([N, OUT]); out[:, lora_ind] = compact.reshape(N, G*OD)

Device strategy (data-parallel over tokens, weights replicated):
  - Host pre-transposes each x shard to fp8-e3m4 xT [D, TPC]: the kernel is
    DMA-bandwidth-bound and x is the dominant input, so e3m4 (4 mantissa
    bits; empirically 1.35e-2 max rel err end-to-end vs the 2e-2 budget)
    halves the x read traffic. The contraction dim d lands on SBUF
    partitions with contiguous 512B DMA lines.
  - A is reordered to feature-major f = (g, e, r) and concatenated with
    W_route^T into one fp16 [128, KD, FE] rhs, pre-arranged on the host so
    each partition's DMA line is fully contiguous (2176B) for full-rate DMA.
    One accumulated matmul chain per 128-token tile produces h (cols
    0..127) and the routing logits (cols 128..135); fp8 lhsT x fp16 rhs is
    a legal mixed-precision matmul.
  - Softmax: exp (no max-subtract; logits are O(1)) with the row-sum fused
    into the ACT instruction via accum_out, one reciprocal, then
    probs = expv * rsum so the final PSUM->SBUF copies are scale-free and
    can run on any engine. SCALING=2 is folded into B on the host.
  - wh = h * probs uses a step-0 broadcast access pattern; wh is
    PE-transposed once per tile and the per-group up-projection runs as
    TWO K=128 matmuls of free-size 1024 against a block-diagonal fp16
    [128, 2048] B (fewer PE-SEQ instructions; PE.SEQ is near-critical).
  - The two [128,1024] fp32->fp16 PSUM drains go to Act and Pool (DVE
    keeps the softmax/wh chain), keeping every engine under the ~1.92us
    per-tile DMA cadence.
  - compact is staged fp16 in SBUF and DMAed out fp16 (halves the dominant
    write); the host upcasts and performs the lora_ind zero-pad scatter
    during unsharding.
"""

import sys
from concurrent.futures import ThreadPoolExecutor
from contextlib import ExitStack

for _p in ("/opt/trn_rl_repo", "/root/.axon_site/_ro/trn_rl_repo"):
    if _p not in sys.path:
        sys.path.insert(0, _p)

import ml_dtypes
import numpy as np

import concourse.bass as bass  # noqa: F401
import concourse.mybir as mybir
import concourse.tile as tile
from concourse import bacc
from concourse.bass_utils import run_bass_kernel_spmd
from concourse.masks import make_identity

# Problem dims (hardcoded per spec nn_MoELoRA_28089086116115)
B, S, D = 4, 4096, 1024
OUT = 3072
R, E, G = 8, 8, 2
OD = OUT // 3                    # 1024
F = G * E * R                    # 128 lora features, f = g*64 + e*8 + r
FE = F + E                       # 136: features + routing logits
SCALING = 16.0 / 8.0
NCORES = 8
NTOK = B * S                     # 16384
TPC = NTOK // NCORES             # 2048 tokens per core
TBLK = 512                       # tokens per x DMA block
NBLK = TPC // TBLK
KD = D // 128                    # 8 contraction chunks

F8 = ml_dtypes.float8_e3m4

# Hooks for test.py (not used by the grader, which calls kernel() only).
_RUN_KWARGS: dict = {}
_LAST: dict = {}

_nc_cache = None


def _build():
    f32 = mybir.dt.float32
    f16 = mybir.dt.float16
    f8 = mybir.dt.float8e3
    Exp = mybir.ActivationFunctionType.Exp
    Copy = mybir.ActivationFunctionType.Copy
    mult = mybir.AluOpType.mult

    nc = bacc.Bacc("TRN2", target_bir_lowering=False, debug=False,
                   num_devices=NCORES)
    xT = nc.dram_tensor("xT", [D, TPC], f8, kind="ExternalInput")
    awt = nc.dram_tensor("AWT", [128, KD, FE], f16, kind="ExternalInput")
    btbd = nc.dram_tensor("BT", [G, E * R, OD], f16, kind="ExternalInput")
    out = nc.dram_tensor("out", [TPC, G * OD], f16, kind="ExternalOutput")

    with tile.TileContext(nc) as tc, ExitStack() as ctx:
        wp = ctx.enter_context(tc.tile_pool(name="wp", bufs=1))
        awt_sb = wp.tile([128, KD, FE], f16)

        bt_sb = wp.tile([128, G * OD], f16)
        nc.gpsimd.memset(bt_sb[:], 0.0)
        ident = wp.tile([128, 128], f16)
        make_identity(nc, ident)



        # all x blocks live in SBUF at once (4 x 4KB/partition, fp8)
        xp = ctx.enter_context(tc.tile_pool(name="xp", bufs=NBLK))
        sp = ctx.enter_context(tc.tile_pool(name="sp", bufs=8))
        outp = ctx.enter_context(tc.tile_pool(name="outp", bufs=5))
        ph = ctx.enter_context(tc.tile_pool(name="ph", bufs=2, space="PSUM"))
        pt = ctx.enter_context(tc.tile_pool(name="pt", bufs=2, space="PSUM"))
        pc = ctx.enter_context(tc.tile_pool(name="pc", bufs=4, space="PSUM"))

        # Warm the PE p-state ramp during the DMA preamble: ~3us of dummy
        # back-to-back transposes (rotating through the pt slots; nothing
        # reads them) bring the tensor engine to full clock before the
        # first real h-matmul arrives — cold-start matmuls otherwise run at
        # 2-4x cycle time for the first 3us of busy.
        warm_ps = pt.tile([128, 128], f16, name="warm", tag="whT_ps")
        for _ in range(30):
            nc.tensor.transpose(warm_ps[:], ident[:], ident[:])

        # weights first (compute needs awt + x block 0), then all x reads
        # up-front so no read ever queues behind a compute-gated write.
        nc.sync.dma_start(awt_sb[:], awt[:])
        x_sbs = []
        for blk in range(NBLK):
            x_sb = xp.tile([128, KD, TBLK], f8, name=f"x{blk}")
            xr = xT[:, blk * TBLK:(blk + 1) * TBLK].rearrange(
                "(k p) t -> p k t", p=128)
            if blk == 0:
                # split block 0 along k so the first h-matmuls start half a
                # block earlier (each k-line stays a full-rate 512B descriptor)
                nc.sync.dma_start(x_sb[:, 0:KD // 2, :], xr[:, 0:KD // 2, :])
                nc.sync.dma_start(x_sb[:, KD // 2:, :], xr[:, KD // 2:, :])
            else:
                nc.sync.dma_start(x_sb[:], xr)
            x_sbs.append(x_sb)
            if blk == 0:
                # BT is block-diagonal: zero the tile (idle Pool engine) and
                # DMA only the two nonzero 128KB blocks.
                nc.sync.dma_start(bt_sb[0:64, 0:1024], btbd[0])
                nc.sync.dma_start(bt_sb[64:128, 1024:2048], btbd[1])

        # Iteration N drains subtile N-1 (emitted at the HEAD of each
        # engine queue so the PSUM cps slots recycle before iteration N's
        # up-proj needs them), transposes/up-projects subtile N, and runs
        # the h-matmuls + softmax for subtile N+2. The two-iteration lead
        # of the softmax keeps the in-order Act/DVE queues from ever
        # delaying the loop-carried transpose chain, so the steady-state
        # cadence is DMA-paced.
        NSUB = TPC // 128
        wh_t = [None] * NSUB       # wh tiles (SBUF fp16), stage S2 output
        whT_t = [None] * NSUB      # whT tiles (SBUF fp16), stage S3 output
        cps_t = [None] * NSUB      # cps PSUM tiles, stage S4 output
        o_sbs = [None] * (NSUB // 2)

        def emit_drains(n):
            """Drain subtile n's four cps banks and issue its output DMA.

            The column split is biased so Act (faster per element, but it
            also runs exp) gets 2.5 banks and DVE 1.5 — both engines land
            just under the DMA cadence.
            """
            half = n % 2
            o_sb = o_sbs[n // 2]
            cl = cps_t[n]
            base = lambda j: j * 512
            # Act: j0, j1, first half of j2
            nc.scalar.activation(o_sb[:, half, 0:512], cl[0][:], Copy)
            nc.scalar.activation(o_sb[:, half, 512:1024], cl[1][:], Copy)
            nc.scalar.activation(o_sb[:, half, 1024:1280], cl[2][:, 0:256],
                                 Copy)
            # DVE: second half of j2, j3
            nc.vector.tensor_copy(o_sb[:, half, 1280:1536], cl[2][:, 256:512])
            nc.vector.tensor_copy(o_sb[:, half, 1536:2048], cl[3][:])
            # output writes: edge pairs go out per-subtile (first writes
            # start a subtile earlier, the final write is half as long);
            # steady-state pairs share one 1 MiB write.
            pair = n // 2
            edge = pair <= 2 or pair >= NSUB // 2 - 2
            r0 = pair * 256
            if n == NSUB - 1:
                # very last write: split by column halves so the first
                # 0.25 MiB goes out while the rest still drains, and the
                # final transfer on the critical tail is half as long
                rows = out[r0 + 128:r0 + 256, :]
                nc.sync.dma_start(rows[:, 0:1024], o_sb[:, 1, 0:1024])
                nc.sync.dma_start(rows[:, 1024:2048], o_sb[:, 1, 1024:2048])
            elif edge:
                nc.sync.dma_start(
                    out[r0 + half * 128:r0 + (half + 1) * 128, :],
                    o_sb[:, half, :])
            elif half == 1:
                nc.sync.dma_start(
                    out[r0:r0 + 256, :].rearrange("(s p) o -> p s o", p=128),
                    o_sb[:])

        # Subtiles whose drains run in the SAME iteration as their up-proj:
        # at the start the Act queue has no backlog and same-iter drains get
        # the write stream going ~2us earlier; at the end they cut the
        # pipeline-lag tail. Steady-state subtiles drain one iteration late
        # so the in-order engine queues never stall the loop-carried chain.
        same_iter = set(range(0, 3)) | set(range(NSUB - 2, NSUB))

        for N in range(-2, NSUB + 1):
            D0 = N - 1   # steady-state: drain + write subtile N-1
            if 0 <= D0 < NSUB and D0 not in same_iter:
                emit_drains(D0)

            if 0 <= N < NSUB:
                # S3: transpose + SBUF copy of subtile N's wh lead the
                # compute section of the PE and DVE queues so the up-proj
                # operand is ready before PE reaches it.
                whT_ps = pt.tile([128, 128], f16)
                nc.tensor.transpose(whT_ps[:], wh_t[N][:], ident[:])
                whT = sp.tile([128, 128], f16, tag="whT")
                nc.vector.tensor_copy(whT[:], whT_ps[:])
                whT_t[N] = whT

            K = N + 2
            if K < NSUB:
                x_sb = x_sbs[K // 4]
                t0 = (K % 4) * 128
                # S1: h (cols 0..127) + routing logits (cols 128..135)
                hE = ph.tile([128, FE], f32)
                for k in range(KD):
                    nc.tensor.matmul(
                        hE[:],
                        lhsT=x_sb[:, k, t0:t0 + 128],
                        rhs=awt_sb[:, k, :],
                        start=(k == 0),
                        stop=(k == KD - 1),
                    )
                # S2: softmax probs = exp(logits) / sum, then wh = h * probs
                expv = sp.tile([128, E], f32, tag="expv")
                ssum = sp.tile([128, 1], f32, tag="ssum")
                # plain exp (no accum_out: the accumulator read costs Act an
                # extra 187ns and Act is the tightest engine); sum on DVE
                nc.scalar.activation(expv[:], hE[:, F:FE], Exp)
                nc.vector.reduce_sum(ssum[:], expv[:],
                                     axis=mybir.AxisListType.X)
                rsum = sp.tile([128, 1], f32, tag="rsum")
                nc.vector.reciprocal(rsum[:], ssum[:])
                probs = sp.tile([128, E], f32, tag="probs")
                nc.gpsimd.tensor_scalar_mul(probs[:], expv[:], rsum[:, 0:1])
                wh = sp.tile([128, F], f16, tag="wh")
                nc.vector.tensor_tensor(
                    out=wh.rearrange("p (g e r) -> p g e r", g=G, e=E),
                    in0=hE[:, 0:F].rearrange("p (g e r) -> p g e r", g=G, e=E),
                    in1=probs[:, None, :, None].to_broadcast([128, G, E, R]),
                    op=mult,
                )
                wh_t[K] = wh

            if 0 <= N < NSUB:
                if N % 2 == 0:
                    o_sbs[N // 2] = outp.tile([128, 2, G * OD], f16,
                                              name=f"o{N // 2}", tag="o")
                # S4: compact[t, (g,o)] via block-diagonal 2*B^T (K=128),
                # one PSUM bank per 512-col matmul; drained next iteration
                cps_l = []
                for j in range(4):
                    cps = pc.tile([128, 512], f32, name=f"cps{j}", tag="cps")
                    nc.tensor.matmul(
                        cps[:],
                        lhsT=whT_t[N][:],
                        rhs=bt_sb[:, j * 512:(j + 1) * 512],
                        start=True,
                        stop=True,
                    )
                    cps_l.append(cps)
                cps_t[N] = cps_l
                if N in same_iter:
                    emit_drains(N)

    nc.compile()
    return nc


def _shard_xT(x, c):
    return np.ascontiguousarray(x[c * TPC:(c + 1) * TPC].T).astype(F8)


_runner = None


def _get_runner(nc):
    """Build the sharded PJRT callable once; reuse across kernel() calls.

    Mirrors bass2jax.run_bass_via_pjrt's multi-core branch, but caches the
    jitted function so repeat calls skip retrace/recompile. Falls back to
    the stock path (handled by caller) on any failure.
    """
    global _runner
    if _runner is not None:
        return _runner
    import jax
    from jax.experimental.shard_map import shard_map
    from jax.sharding import Mesh, PartitionSpec

    from concourse import bass2jax, mybir as _mb

    bass2jax.install_neuronx_cc_hook()
    partition_name = (nc.partition_id_tensor.name
                      if nc.partition_id_tensor else None)
    in_names, out_names, out_avals = [], [], []
    for alloc in nc.m.functions[0].allocations:
        if not isinstance(alloc, _mb.MemoryLocationSet):
            continue
        name = alloc.memorylocations[0].name
        if alloc.kind == "ExternalInput":
            if name != partition_name:
                in_names.append(name)
        elif alloc.kind == "ExternalOutput":
            out_names.append(name)
            out_avals.append(jax.core.ShapedArray(
                tuple(alloc.tensor_shape), _mb.dt.np(alloc.dtype)))
    n_params = len(in_names)
    n_outs = len(out_avals)
    all_in_names = list(in_names) + list(out_names)
    if partition_name is not None:
        all_in_names.append(partition_name)

    def _body(*args):
        operands = list(args)
        if partition_name is not None:
            operands.append(bass2jax.partition_id_tensor())
        outs = bass2jax._bass_exec_p.bind(
            *operands,
            out_avals=tuple(out_avals),
            in_names=tuple(all_in_names),
            out_names=tuple(out_names),
            lowering_input_output_aliases=(),
            sim_require_finite=True,
            sim_require_nnan=True,
            nc=nc,
        )
        return tuple(outs)

    devices = jax.devices()[:NCORES]
    mesh = Mesh(np.asarray(devices), ("core",))
    specs = (PartitionSpec("core"),) * (n_params + n_outs)
    sharded = jax.jit(
        shard_map(_body, mesh=mesh, in_specs=specs,
                  out_specs=(PartitionSpec("core"),) * n_outs,
                  check_rep=False),
        donate_argnums=tuple(range(n_params, n_params + n_outs)),
        keep_unused=True,
    )
    _runner = (sharded, in_names, out_names, out_avals)
    return _runner


def _run_cached(nc, in_maps):
    sharded, in_names, out_names, out_avals = _get_runner(nc)
    concat_in = [
        np.concatenate([np.asarray(m[name]) for m in in_maps], axis=0)
        for name in in_names
    ]
    concat_zeros = [
        np.zeros((NCORES * a.shape[0], *a.shape[1:]), a.dtype)
        for a in out_avals
    ]
    out_arrs = sharded(*concat_in, *concat_zeros)
    return [
        {name: np.asarray(out_arrs[i]).reshape(NCORES, *out_avals[i].shape)[c]
         for i, name in enumerate(out_names)}
        for c in range(NCORES)
    ]


def kernel(x, W_route, A, Bw, lora_ind):
    global _nc_cache
    x = np.asarray(x, dtype=np.float32).reshape(NTOK, D)
    W_route = np.asarray(W_route, dtype=np.float32)
    A = np.asarray(A, dtype=np.float32)
    Bw = np.asarray(Bw, dtype=np.float32)
    lora_ind = np.asarray(lora_ind).astype(np.int64)

    # [D, 136] fp16: cols 0..127 are A rows in (g, e, r) order, 128.. W_route;
    # re-packed to [128, KD, FE] so each partition's DMA line is contiguous.
    A_all = A.transpose(1, 0, 2, 3).reshape(F, D)
    AWT = np.concatenate([A_all.T, W_route.T], axis=1).astype(np.float16)
    AWT_dev = np.ascontiguousarray(
        AWT.reshape(KD, 128, FE).transpose(1, 0, 2))
    # block-diagonal B^T with SCALING folded in: rows (g,e,r), cols (g,o)
    BTbd = (Bw.transpose(1, 0, 3, 2).reshape(G, E * R, OD)
            * SCALING).astype(np.float16)

    if _nc_cache is None:
        _nc_cache = _build()
    nc = _nc_cache

    with ThreadPoolExecutor(NCORES) as ex:
        xTs = list(ex.map(lambda c: _shard_xT(x, c), range(NCORES)))
    in_maps = [{"xT": xTs[c], "AWT": AWT_dev, "BT": BTbd}
               for c in range(NCORES)]

    try:
        results = _run_cached(nc, in_maps)
    except Exception:  # noqa: BLE001  (fall back to the stock SPMD path)
        global _runner
        _runner = None
        res = run_bass_kernel_spmd(nc, in_maps, core_ids=list(range(NCORES)),
                                   **_RUN_KWARGS)
        results = res.results
    _LAST["results"] = results

    compact = np.concatenate(
        [results[c]["out"] for c in range(NCORES)], axis=0)
    outp = np.zeros((NTOK, OUT), dtype=np.float32)
    outp[:, lora_ind] = compact.astype(np.float32)
    return outp.reshape(B, S, OUT)


# revision 28
# speedup vs baseline: 1.0431x; 1.0114x over previous
"""MoELoRA forward kernel for 8x Trainium2 NeuronCores (Bass/Tile).

Math (see reference):
  route   = softmax(x @ W_route^T)                      [N, E]
  h       = x @ A[e,g,r,:]^T                            [N, E, G, R]
  wh      = h * route[..., None, None]
  compact = einsum(wh, Bw[e,g,o,r]) * SCALING           [N, G, OD]
  out     = zeros([N, OUT]); out[:, lora_ind] = compact.reshape(N, G*OD)

Device strategy (data-parallel over tokens, weights replicated):
  - Host pre-transposes each x shard to fp8-e3m4 xT [D, TPC]: the kernel is
    DMA-bandwidth-bound and x is the dominant input, so e3m4 (4 mantissa
    bits; empirically 1.35e-2 max rel err end-to-end vs the 2e-2 budget)
    halves the x read traffic. The contraction dim d lands on SBUF
    partitions with contiguous 512B DMA lines.
  - A is reordered to feature-major f = (g, e, r) and concatenated with
    W_route^T into one fp16 [128, KD, FE] rhs, pre-arranged on the host so
    each partition's DMA line is fully contiguous (2176B) for full-rate DMA.
    One accumulated matmul chain per 128-token tile produces h (cols
    0..127) and the routing logits (cols 128..135); fp8 lhsT x fp16 rhs is
    a legal mixed-precision matmul.
  - Softmax: exp (no max-subtract; logits are O(1)) with the row-sum fused
    into the ACT instruction via accum_out, one reciprocal, then
    probs = expv * rsum so the final PSUM->SBUF copies are scale-free and
    can run on any engine. SCALING=2 is folded into B on the host.
  - wh = h * probs uses a step-0 broadcast access pattern; wh is
    PE-transposed once per tile and the per-group up-projection runs as
    TWO K=128 matmuls of free-size 1024 against a block-diagonal fp16
    [128, 2048] B (fewer PE-SEQ instructions; PE.SEQ is near-critical).
  - The two [128,1024] fp32->fp16 PSUM drains go to Act and Pool (DVE
    keeps the softmax/wh chain), keeping every engine under the ~1.92us
    per-tile DMA cadence.
  - compact is staged fp16 in SBUF and DMAed out fp16 (halves the dominant
    write); the host upcasts and performs the lora_ind zero-pad scatter
    during unsharding.
"""

import sys
from concurrent.futures import ThreadPoolExecutor
from contextlib import ExitStack

for _p in ("/opt/trn_rl_repo", "/root/.axon_site/_ro/trn_rl_repo"):
    if _p not in sys.path:
        sys.path.insert(0, _p)

import ml_dtypes
import numpy as np

import concourse.bass as bass  # noqa: F401
import concourse.mybir as mybir
import concourse.tile as tile
from concourse import bacc
from concourse.bass_utils import run_bass_kernel_spmd
from concourse.masks import make_identity

# Problem dims (hardcoded per spec nn_MoELoRA_28089086116115)
B, S, D = 4, 4096, 1024
OUT = 3072
R, E, G = 8, 8, 2
OD = OUT // 3                    # 1024
F = G * E * R                    # 128 lora features, f = g*64 + e*8 + r
FE = F + E                       # 136: features + routing logits
SCALING = 16.0 / 8.0
NCORES = 8
NTOK = B * S                     # 16384
TPC = NTOK // NCORES             # 2048 tokens per core
TBLK = 512                       # tokens per x DMA block
NBLK = TPC // TBLK
KD = D // 128                    # 8 contraction chunks

F8 = ml_dtypes.float8_e3m4

# Hooks for test.py (not used by the grader, which calls kernel() only).
_RUN_KWARGS: dict = {}
_LAST: dict = {}

_nc_cache = None


def _build():
    f32 = mybir.dt.float32
    f16 = mybir.dt.float16
    f8 = mybir.dt.float8e3
    Exp = mybir.ActivationFunctionType.Exp
    Copy = mybir.ActivationFunctionType.Copy
    mult = mybir.AluOpType.mult

    nc = bacc.Bacc("TRN2", target_bir_lowering=False, debug=False,
                   num_devices=NCORES)
    xT = nc.dram_tensor("xT", [D, TPC], f8, kind="ExternalInput")
    awt = nc.dram_tensor("AWT", [128, KD, FE], f16, kind="ExternalInput")
    btbd = nc.dram_tensor("BT", [G, E * R, OD], f16, kind="ExternalInput")
    out = nc.dram_tensor("out", [TPC, G * OD], f16, kind="ExternalOutput")

    with tile.TileContext(nc) as tc, ExitStack() as ctx:
        wp = ctx.enter_context(tc.tile_pool(name="wp", bufs=1))
        awt_sb = wp.tile([128, KD, FE], f16)

        bt_sb = wp.tile([128, G * OD], f16)
        nc.gpsimd.memset(bt_sb[:], 0.0)
        ident = wp.tile([128, 128], f16)
        make_identity(nc, ident)



        # all x blocks live in SBUF at once (4 x 4KB/partition, fp8)
        xp = ctx.enter_context(tc.tile_pool(name="xp", bufs=NBLK))
        sp = ctx.enter_context(tc.tile_pool(name="sp", bufs=8))
        outp = ctx.enter_context(tc.tile_pool(name="outp", bufs=5))
        ph = ctx.enter_context(tc.tile_pool(name="ph", bufs=2, space="PSUM"))
        pt = ctx.enter_context(tc.tile_pool(name="pt", bufs=2, space="PSUM"))
        pc = ctx.enter_context(tc.tile_pool(name="pc", bufs=4, space="PSUM"))

        # Warm the PE p-state ramp during the DMA preamble: ~3us of dummy
        # back-to-back transposes (rotating through the pt slots; nothing
        # reads them) bring the tensor engine to full clock before the
        # first real h-matmul arrives — cold-start matmuls otherwise run at
        # 2-4x cycle time for the first 3us of busy.
        warm_ps = pt.tile([128, 128], f16, name="warm", tag="whT_ps")
        for _ in range(30):
            nc.tensor.transpose(warm_ps[:], ident[:], ident[:])

        # weights first (compute needs awt + x block 0), then all x reads
        # up-front so no read ever queues behind a compute-gated write.
        nc.sync.dma_start(awt_sb[:], awt[:])
        x_sbs = []
        for blk in range(NBLK):
            x_sb = xp.tile([128, KD, TBLK], f8, name=f"x{blk}")
            xr = xT[:, blk * TBLK:(blk + 1) * TBLK].rearrange(
                "(k p) t -> p k t", p=128)
            if blk == 0:
                # split block 0 along k so the first h-matmuls start half a
                # block earlier (each k-line stays a full-rate 512B descriptor)
                nc.sync.dma_start(x_sb[:, 0:KD // 2, :], xr[:, 0:KD // 2, :])
                nc.sync.dma_start(x_sb[:, KD // 2:, :], xr[:, KD // 2:, :])
            else:
                nc.sync.dma_start(x_sb[:], xr)
            x_sbs.append(x_sb)
            if blk == 0:
                # BT is block-diagonal: zero the tile (idle Pool engine) and
                # DMA only the two nonzero 128KB blocks.
                nc.sync.dma_start(bt_sb[0:64, 0:1024], btbd[0])
                nc.sync.dma_start(bt_sb[64:128, 1024:2048], btbd[1])

        # Iteration N drains subtile N-1 (emitted at the HEAD of each
        # engine queue so the PSUM cps slots recycle before iteration N's
        # up-proj needs them), transposes/up-projects subtile N, and runs
        # the h-matmuls + softmax for subtile N+2. The two-iteration lead
        # of the softmax keeps the in-order Act/DVE queues from ever
        # delaying the loop-carried transpose chain, so the steady-state
        # cadence is DMA-paced.
        NSUB = TPC // 128
        wh_t = [None] * NSUB       # wh tiles (SBUF fp16), stage S2 output
        whT_t = [None] * NSUB      # whT tiles (SBUF fp16), stage S3 output
        cps_t = [None] * NSUB      # cps PSUM tiles, stage S4 output
        o_sbs = [None] * (NSUB // 2)

        def emit_drains(n):
            """Drain subtile n's four cps banks and issue its output DMA.

            The column split is biased so Act (faster per element, but it
            also runs exp) gets 2.5 banks and DVE 1.5 — both engines land
            just under the DMA cadence.
            """
            half = n % 2
            o_sb = o_sbs[n // 2]
            cl = cps_t[n]
            # Act: j0, j1; DVE: j2, j3 (with whTcopy moved to Act both
            # engines land at ~1.71us/subtile, just under the DMA cadence)
            nc.scalar.activation(o_sb[:, half, 0:512], cl[0][:], Copy)
            nc.scalar.activation(o_sb[:, half, 512:1024], cl[1][:], Copy)
            nc.vector.tensor_copy(o_sb[:, half, 1024:1536], cl[2][:])
            nc.vector.tensor_copy(o_sb[:, half, 1536:2048], cl[3][:])
            # output writes: edge pairs go out per-subtile (first writes
            # start a subtile earlier, the final write is half as long);
            # steady-state pairs share one 1 MiB write.
            pair = n // 2
            edge = pair <= 2 or pair >= NSUB // 2 - 2
            r0 = pair * 256
            if n == NSUB - 1:
                # very last write: split by column halves so the first
                # 0.25 MiB goes out while the rest still drains, and the
                # final transfer on the critical tail is half as long
                rows = out[r0 + 128:r0 + 256, :]
                nc.sync.dma_start(rows[:, 0:1024], o_sb[:, 1, 0:1024])
                nc.sync.dma_start(rows[:, 1024:2048], o_sb[:, 1, 1024:2048])
            elif edge:
                nc.sync.dma_start(
                    out[r0 + half * 128:r0 + (half + 1) * 128, :],
                    o_sb[:, half, :])
            elif half == 1:
                nc.sync.dma_start(
                    out[r0:r0 + 256, :].rearrange("(s p) o -> p s o", p=128),
                    o_sb[:])

        # Subtiles whose drains run in the SAME iteration as their up-proj:
        # at the start the Act queue has no backlog and same-iter drains get
        # the write stream going ~2us earlier; at the end they cut the
        # pipeline-lag tail. Steady-state subtiles drain one iteration late
        # so the in-order engine queues never stall the loop-carried chain.
        same_iter = set(range(0, 3)) | set(range(NSUB - 2, NSUB))

        for N in range(-2, NSUB + 1):
            D0 = N - 1   # steady-state: drain + write subtile N-1
            if 0 <= D0 < NSUB and D0 not in same_iter:
                emit_drains(D0)

            if 0 <= N < NSUB:
                # S3: transpose + SBUF copy of subtile N's wh lead the
                # compute section of the PE and DVE queues so the up-proj
                # operand is ready before PE reaches it.
                whT_ps = pt.tile([128, 128], f16)
                nc.tensor.transpose(whT_ps[:], wh_t[N][:], ident[:])
                whT = sp.tile([128, 128], f16, tag="whT")
                # whT copy on Act (not DVE): after the drain rebalance DVE
                # carries the softmax chain + two 512-col drains
                nc.scalar.activation(whT[:], whT_ps[:], Copy)
                whT_t[N] = whT

            K = N + 2
            if K < NSUB:
                x_sb = x_sbs[K // 4]
                t0 = (K % 4) * 128
                # S1: h (cols 0..127) + routing logits (cols 128..135)
                hE = ph.tile([128, FE], f32)
                for k in range(KD):
                    nc.tensor.matmul(
                        hE[:],
                        lhsT=x_sb[:, k, t0:t0 + 128],
                        rhs=awt_sb[:, k, :],
                        start=(k == 0),
                        stop=(k == KD - 1),
                    )
                # S2: softmax probs = exp(logits) / sum, then wh = h * probs
                expv = sp.tile([128, E], f32, tag="expv")
                ssum = sp.tile([128, 1], f32, tag="ssum")
                # plain exp (no accum_out: the accumulator read costs Act an
                # extra 187ns and Act is the tightest engine); sum on DVE
                nc.scalar.activation(expv[:], hE[:, F:FE], Exp)
                nc.vector.reduce_sum(ssum[:], expv[:],
                                     axis=mybir.AxisListType.X)
                rsum = sp.tile([128, 1], f32, tag="rsum")
                nc.vector.reciprocal(rsum[:], ssum[:])
                probs = sp.tile([128, E], f32, tag="probs")
                nc.gpsimd.tensor_scalar_mul(probs[:], expv[:], rsum[:, 0:1])
                wh = sp.tile([128, F], f16, tag="wh")
                nc.vector.tensor_tensor(
                    out=wh.rearrange("p (g e r) -> p g e r", g=G, e=E),
                    in0=hE[:, 0:F].rearrange("p (g e r) -> p g e r", g=G, e=E),
                    in1=probs[:, None, :, None].to_broadcast([128, G, E, R]),
                    op=mult,
                )
                wh_t[K] = wh

            if 0 <= N < NSUB:
                if N % 2 == 0:
                    o_sbs[N // 2] = outp.tile([128, 2, G * OD], f16,
                                              name=f"o{N // 2}", tag="o")
                # S4: compact[t, (g,o)] via block-diagonal 2*B^T (K=128),
                # one PSUM bank per 512-col matmul; drained next iteration
                cps_l = []
                for j in range(4):
                    cps = pc.tile([128, 512], f32, name=f"cps{j}", tag="cps")
                    nc.tensor.matmul(
                        cps[:],
                        lhsT=whT_t[N][:],
                        rhs=bt_sb[:, j * 512:(j + 1) * 512],
                        start=True,
                        stop=True,
                    )
                    cps_l.append(cps)
                cps_t[N] = cps_l
                if N in same_iter:
                    emit_drains(N)

    nc.compile()
    return nc


def _shard_xT(x, c):
    return np.ascontiguousarray(x[c * TPC:(c + 1) * TPC].T).astype(F8)


_runner = None


def _get_runner(nc):
    """Build the sharded PJRT callable once; reuse across kernel() calls.

    Mirrors bass2jax.run_bass_via_pjrt's multi-core branch, but caches the
    jitted function so repeat calls skip retrace/recompile. Falls back to
    the stock path (handled by caller) on any failure.
    """
    global _runner
    if _runner is not None:
        return _runner
    import jax
    from jax.experimental.shard_map import shard_map
    from jax.sharding import Mesh, PartitionSpec

    from concourse import bass2jax, mybir as _mb

    bass2jax.install_neuronx_cc_hook()
    partition_name = (nc.partition_id_tensor.name
                      if nc.partition_id_tensor else None)
    in_names, out_names, out_avals = [], [], []
    for alloc in nc.m.functions[0].allocations:
        if not isinstance(alloc, _mb.MemoryLocationSet):
            continue
        name = alloc.memorylocations[0].name
        if alloc.kind == "ExternalInput":
            if name != partition_name:
                in_names.append(name)
        elif alloc.kind == "ExternalOutput":
            out_names.append(name)
            out_avals.append(jax.core.ShapedArray(
                tuple(alloc.tensor_shape), _mb.dt.np(alloc.dtype)))
    n_params = len(in_names)
    n_outs = len(out_avals)
    all_in_names = list(in_names) + list(out_names)
    if partition_name is not None:
        all_in_names.append(partition_name)

    def _body(*args):
        operands = list(args)
        if partition_name is not None:
            operands.append(bass2jax.partition_id_tensor())
        outs = bass2jax._bass_exec_p.bind(
            *operands,
            out_avals=tuple(out_avals),
            in_names=tuple(all_in_names),
            out_names=tuple(out_names),
            lowering_input_output_aliases=(),
            sim_require_finite=True,
            sim_require_nnan=True,
            nc=nc,
        )
        return tuple(outs)

    devices = jax.devices()[:NCORES]
    mesh = Mesh(np.asarray(devices), ("core",))
    specs = (PartitionSpec("core"),) * (n_params + n_outs)
    sharded = jax.jit(
        shard_map(_body, mesh=mesh, in_specs=specs,
                  out_specs=(PartitionSpec("core"),) * n_outs,
                  check_rep=False),
        donate_argnums=tuple(range(n_params, n_params + n_outs)),
        keep_unused=True,
    )
    _runner = (sharded, in_names, out_names, out_avals)
    return _runner


def _run_cached(nc, in_maps):
    sharded, in_names, out_names, out_avals = _get_runner(nc)
    concat_in = [
        np.concatenate([np.asarray(m[name]) for m in in_maps], axis=0)
        for name in in_names
    ]
    concat_zeros = [
        np.zeros((NCORES * a.shape[0], *a.shape[1:]), a.dtype)
        for a in out_avals
    ]
    out_arrs = sharded(*concat_in, *concat_zeros)
    return [
        {name: np.asarray(out_arrs[i]).reshape(NCORES, *out_avals[i].shape)[c]
         for i, name in enumerate(out_names)}
        for c in range(NCORES)
    ]


def kernel(x, W_route, A, Bw, lora_ind):
    global _nc_cache
    x = np.asarray(x, dtype=np.float32).reshape(NTOK, D)
    W_route = np.asarray(W_route, dtype=np.float32)
    A = np.asarray(A, dtype=np.float32)
    Bw = np.asarray(Bw, dtype=np.float32)
    lora_ind = np.asarray(lora_ind).astype(np.int64)

    # [D, 136] fp16: cols 0..127 are A rows in (g, e, r) order, 128.. W_route;
    # re-packed to [128, KD, FE] so each partition's DMA line is contiguous.
    A_all = A.transpose(1, 0, 2, 3).reshape(F, D)
    AWT = np.concatenate([A_all.T, W_route.T], axis=1).astype(np.float16)
    AWT_dev = np.ascontiguousarray(
        AWT.reshape(KD, 128, FE).transpose(1, 0, 2))
    # block-diagonal B^T with SCALING folded in: rows (g,e,r), cols (g,o)
    BTbd = (Bw.transpose(1, 0, 3, 2).reshape(G, E * R, OD)
            * SCALING).astype(np.float16)

    if _nc_cache is None:
        _nc_cache = _build()
    nc = _nc_cache

    with ThreadPoolExecutor(NCORES) as ex:
        xTs = list(ex.map(lambda c: _shard_xT(x, c), range(NCORES)))
    in_maps = [{"xT": xTs[c], "AWT": AWT_dev, "BT": BTbd}
               for c in range(NCORES)]

    try:
        results = _run_cached(nc, in_maps)
    except Exception:  # noqa: BLE001  (fall back to the stock SPMD path)
        global _runner
        _runner = None
        res = run_bass_kernel_spmd(nc, in_maps, core_ids=list(range(NCORES)),
                                   **_RUN_KWARGS)
        results = res.results
    _LAST["results"] = results

    compact = np.concatenate(
        [results[c]["out"] for c in range(NCORES)], axis=0)
    outp = np.zeros((NTOK, OUT), dtype=np.float32)
    outp[:, lora_ind] = compact.astype(np.float32)
    return outp.reshape(B, S, OUT)


# revision 29
# speedup vs baseline: 1.0440x; 1.0008x over previous
"""MoELoRA forward kernel for 8x Trainium2 NeuronCores (Bass/Tile).

Math (see reference):
  route   = softmax(x @ W_route^T)                      [N, E]
  h       = x @ A[e,g,r,:]^T                            [N, E, G, R]
  wh      = h * route[..., None, None]
  compact = einsum(wh, Bw[e,g,o,r]) * SCALING           [N, G, OD]
  out     = zeros([N, OUT]); out[:, lora_ind] = compact.reshape(N, G*OD)

Device strategy (data-parallel over tokens, weights replicated):
  - Host pre-transposes each x shard to fp8-e3m4 xT [D, TPC]: the kernel is
    DMA-bandwidth-bound and x is the dominant input, so e3m4 (4 mantissa
    bits; empirically 1.35e-2 max rel err end-to-end vs the 2e-2 budget)
    halves the x read traffic. The contraction dim d lands on SBUF
    partitions with contiguous 512B DMA lines.
  - A is reordered to feature-major f = (g, e, r) and concatenated with
    W_route^T into one fp16 [128, KD, FE] rhs, pre-arranged on the host so
    each partition's DMA line is fully contiguous (2176B) for full-rate DMA.
    One accumulated matmul chain per 128-token tile produces h (cols
    0..127) and the routing logits (cols 128..135); fp8 lhsT x fp16 rhs is
    a legal mixed-precision matmul.
  - Softmax: exp (no max-subtract; logits are O(1)) with the row-sum fused
    into the ACT instruction via accum_out, one reciprocal, then
    probs = expv * rsum so the final PSUM->SBUF copies are scale-free and
    can run on any engine. SCALING=2 is folded into B on the host.
  - wh = h * probs uses a step-0 broadcast access pattern; wh is
    PE-transposed once per tile and the per-group up-projection runs as
    TWO K=128 matmuls of free-size 1024 against a block-diagonal fp16
    [128, 2048] B (fewer PE-SEQ instructions; PE.SEQ is near-critical).
  - The two [128,1024] fp32->fp16 PSUM drains go to Act and Pool (DVE
    keeps the softmax/wh chain), keeping every engine under the ~1.92us
    per-tile DMA cadence.
  - compact is staged fp16 in SBUF and DMAed out fp16 (halves the dominant
    write); the host upcasts and performs the lora_ind zero-pad scatter
    during unsharding.
"""

import sys
from concurrent.futures import ThreadPoolExecutor
from contextlib import ExitStack

for _p in ("/opt/trn_rl_repo", "/root/.axon_site/_ro/trn_rl_repo"):
    if _p not in sys.path:
        sys.path.insert(0, _p)

import ml_dtypes
import numpy as np

import concourse.bass as bass  # noqa: F401
import concourse.mybir as mybir
import concourse.tile as tile
from concourse import bacc
from concourse.bass_utils import run_bass_kernel_spmd
from concourse.masks import make_identity

# Problem dims (hardcoded per spec nn_MoELoRA_28089086116115)
B, S, D = 4, 4096, 1024
OUT = 3072
R, E, G = 8, 8, 2
OD = OUT // 3                    # 1024
F = G * E * R                    # 128 lora features, f = g*64 + e*8 + r
FE = F + E                       # 136: features + routing logits
SCALING = 16.0 / 8.0
NCORES = 8
NTOK = B * S                     # 16384
TPC = NTOK // NCORES             # 2048 tokens per core
TBLK = 512                       # tokens per x DMA block
NBLK = TPC // TBLK
KD = D // 128                    # 8 contraction chunks

F8 = ml_dtypes.float8_e3m4

# Hooks for test.py (not used by the grader, which calls kernel() only).
_RUN_KWARGS: dict = {}
_LAST: dict = {}

_nc_cache = None


def _build():
    f32 = mybir.dt.float32
    f16 = mybir.dt.float16
    f8 = mybir.dt.float8e3
    Exp = mybir.ActivationFunctionType.Exp
    Copy = mybir.ActivationFunctionType.Copy
    mult = mybir.AluOpType.mult

    nc = bacc.Bacc("TRN2", target_bir_lowering=False, debug=False,
                   num_devices=NCORES)
    xT = nc.dram_tensor("xT", [D, TPC], f8, kind="ExternalInput")
    awt = nc.dram_tensor("AWT", [128, KD, FE], f16, kind="ExternalInput")
    btbd = nc.dram_tensor("BT", [G, E * R, OD], f16, kind="ExternalInput")
    out = nc.dram_tensor("out", [TPC, G * OD], f16, kind="ExternalOutput")

    with tile.TileContext(nc) as tc, ExitStack() as ctx:
        wp = ctx.enter_context(tc.tile_pool(name="wp", bufs=1))
        awt_sb = wp.tile([128, KD, FE], f16)

        bt_sb = wp.tile([128, G * OD], f16)
        nc.gpsimd.memset(bt_sb[:], 0.0)
        ident = wp.tile([128, 128], f16)
        make_identity(nc, ident)



        # all x blocks live in SBUF at once (4 x 4KB/partition, fp8)
        xp = ctx.enter_context(tc.tile_pool(name="xp", bufs=NBLK))
        sp = ctx.enter_context(tc.tile_pool(name="sp", bufs=8))
        outp = ctx.enter_context(tc.tile_pool(name="outp", bufs=5))
        ph = ctx.enter_context(tc.tile_pool(name="ph", bufs=2, space="PSUM"))
        pt = ctx.enter_context(tc.tile_pool(name="pt", bufs=2, space="PSUM"))
        pc = ctx.enter_context(tc.tile_pool(name="pc", bufs=4, space="PSUM"))

        # Warm the PE p-state ramp during the DMA preamble: ~3us of dummy
        # back-to-back transposes (rotating through the pt slots; nothing
        # reads them) bring the tensor engine to full clock before the
        # first real h-matmul arrives — cold-start matmuls otherwise run at
        # 2-4x cycle time for the first 3us of busy.
        warm_ps = pt.tile([128, 128], f16, name="warm", tag="whT_ps")
        for _ in range(30):
            nc.tensor.transpose(warm_ps[:], ident[:], ident[:])

        # weights first (compute needs awt + x block 0), then all x reads
        # up-front so no read ever queues behind a compute-gated write.
        nc.sync.dma_start(awt_sb[:], awt[:])
        x_sbs = []
        for blk in range(NBLK):
            x_sb = xp.tile([128, KD, TBLK], f8, name=f"x{blk}")
            xr = xT[:, blk * TBLK:(blk + 1) * TBLK].rearrange(
                "(k p) t -> p k t", p=128)
            if blk == 0:
                # split block 0 along k so the first h-matmuls start half a
                # block earlier (each k-line stays a full-rate 512B descriptor)
                nc.sync.dma_start(x_sb[:, 0:KD // 2, :], xr[:, 0:KD // 2, :])
                nc.sync.dma_start(x_sb[:, KD // 2:, :], xr[:, KD // 2:, :])
            else:
                nc.sync.dma_start(x_sb[:], xr)
            x_sbs.append(x_sb)
            if blk == 0:
                # BT is block-diagonal: zero the tile (idle Pool engine) and
                # DMA only the two nonzero 128KB blocks.
                nc.sync.dma_start(bt_sb[0:64, 0:1024], btbd[0])
                nc.sync.dma_start(bt_sb[64:128, 1024:2048], btbd[1])

        # Iteration N drains subtile N-1 (emitted at the HEAD of each
        # engine queue so the PSUM cps slots recycle before iteration N's
        # up-proj needs them), transposes/up-projects subtile N, and runs
        # the h-matmuls + softmax for subtile N+2. The two-iteration lead
        # of the softmax keeps the in-order Act/DVE queues from ever
        # delaying the loop-carried transpose chain, so the steady-state
        # cadence is DMA-paced.
        NSUB = TPC // 128
        wh_t = [None] * NSUB       # wh tiles (SBUF fp16), stage S2 output
        whT_t = [None] * NSUB      # whT tiles (SBUF fp16), stage S3 output
        cps_t = [None] * NSUB      # cps PSUM tiles, stage S4 output
        o_sbs = [None] * (NSUB // 2)

        def emit_drains(n):
            """Drain subtile n's four cps banks and issue its output DMA.

            The column split is biased so Act (faster per element, but it
            also runs exp) gets 2.5 banks and DVE 1.5 — both engines land
            just under the DMA cadence.
            """
            half = n % 2
            o_sb = o_sbs[n // 2]
            cl = cps_t[n]
            # Act: j0, j1; DVE: j2, j3 (with whTcopy moved to Act both
            # engines land at ~1.71us/subtile, just under the DMA cadence)
            nc.scalar.activation(o_sb[:, half, 0:512], cl[0][:], Copy)
            nc.scalar.activation(o_sb[:, half, 512:1024], cl[1][:], Copy)
            nc.vector.tensor_copy(o_sb[:, half, 1024:1536], cl[2][:])
            nc.vector.tensor_copy(o_sb[:, half, 1536:2048], cl[3][:])
            # output writes: edge pairs go out per-subtile (first writes
            # start a subtile earlier, the final write is half as long);
            # steady-state pairs share one 1 MiB write.
            pair = n // 2
            edge = pair <= 2 or pair >= NSUB // 2 - 2
            r0 = pair * 256
            if n == NSUB - 1:
                # very last write: split by column halves so the first
                # 0.25 MiB goes out while the rest still drains, and the
                # final transfer on the critical tail is half as long
                rows = out[r0 + 128:r0 + 256, :]
                nc.sync.dma_start(rows[:, 0:1024], o_sb[:, 1, 0:1024])
                nc.sync.dma_start(rows[:, 1024:2048], o_sb[:, 1, 1024:2048])
            elif edge:
                nc.sync.dma_start(
                    out[r0 + half * 128:r0 + (half + 1) * 128, :],
                    o_sb[:, half, :])
            elif half == 1:
                nc.sync.dma_start(
                    out[r0:r0 + 256, :].rearrange("(s p) o -> p s o", p=128),
                    o_sb[:])

        # Subtiles whose drains run in the SAME iteration as their up-proj:
        # at the start the Act queue has no backlog and same-iter drains get
        # the write stream going ~2us earlier; at the end they cut the
        # pipeline-lag tail. Steady-state subtiles drain one iteration late
        # so the in-order engine queues never stall the loop-carried chain.
        same_iter = set(range(0, 3)) | set(range(NSUB - 2, NSUB))

        for N in range(-2, NSUB + 1):
            if 0 <= N < NSUB:
                # S3 first: the transpose + SBUF copy of subtile N's wh lead
                # every engine queue (ahead of the drains) so the up-proj
                # operand is ready before PE reaches it.
                whT_ps = pt.tile([128, 128], f16)
                nc.tensor.transpose(whT_ps[:], wh_t[N][:], ident[:])
                whT = sp.tile([128, 128], f16, tag="whT")
                # whT copy on Act (not DVE): after the drain rebalance DVE
                # carries the softmax chain + two 512-col drains
                nc.scalar.activation(whT[:], whT_ps[:], Copy)
                whT_t[N] = whT

            D0 = N - 1   # steady-state: drain + write subtile N-1
            if 0 <= D0 < NSUB and D0 not in same_iter:
                emit_drains(D0)

            K = N + 2
            if K < NSUB:
                x_sb = x_sbs[K // 4]
                t0 = (K % 4) * 128
                # S1: h (cols 0..127) + routing logits (cols 128..135)
                hE = ph.tile([128, FE], f32)
                for k in range(KD):
                    nc.tensor.matmul(
                        hE[:],
                        lhsT=x_sb[:, k, t0:t0 + 128],
                        rhs=awt_sb[:, k, :],
                        start=(k == 0),
                        stop=(k == KD - 1),
                    )
                # S2: softmax probs = exp(logits) / sum, then wh = h * probs
                expv = sp.tile([128, E], f32, tag="expv")
                ssum = sp.tile([128, 1], f32, tag="ssum")
                # plain exp (no accum_out: the accumulator read costs Act an
                # extra 187ns and Act is the tightest engine); sum on DVE
                nc.scalar.activation(expv[:], hE[:, F:FE], Exp)
                nc.vector.reduce_sum(ssum[:], expv[:],
                                     axis=mybir.AxisListType.X)
                rsum = sp.tile([128, 1], f32, tag="rsum")
                nc.vector.reciprocal(rsum[:], ssum[:])
                probs = sp.tile([128, E], f32, tag="probs")
                nc.gpsimd.tensor_scalar_mul(probs[:], expv[:], rsum[:, 0:1])
                wh = sp.tile([128, F], f16, tag="wh")
                nc.vector.tensor_tensor(
                    out=wh.rearrange("p (g e r) -> p g e r", g=G, e=E),
                    in0=hE[:, 0:F].rearrange("p (g e r) -> p g e r", g=G, e=E),
                    in1=probs[:, None, :, None].to_broadcast([128, G, E, R]),
                    op=mult,
                )
                wh_t[K] = wh

            if 0 <= N < NSUB:
                if N % 2 == 0:
                    o_sbs[N // 2] = outp.tile([128, 2, G * OD], f16,
                                              name=f"o{N // 2}", tag="o")
                # S4: compact[t, (g,o)] via block-diagonal 2*B^T (K=128),
                # one PSUM bank per 512-col matmul; drained next iteration
                cps_l = []
                for j in range(4):
                    cps = pc.tile([128, 512], f32, name=f"cps{j}", tag="cps")
                    nc.tensor.matmul(
                        cps[:],
                        lhsT=whT_t[N][:],
                        rhs=bt_sb[:, j * 512:(j + 1) * 512],
                        start=True,
                        stop=True,
                    )
                    cps_l.append(cps)
                cps_t[N] = cps_l
                if N in same_iter:
                    emit_drains(N)

    nc.compile()
    return nc


def _shard_xT(x, c):
    return np.ascontiguousarray(x[c * TPC:(c + 1) * TPC].T).astype(F8)


_runner = None


def _get_runner(nc):
    """Build the sharded PJRT callable once; reuse across kernel() calls.

    Mirrors bass2jax.run_bass_via_pjrt's multi-core branch, but caches the
    jitted function so repeat calls skip retrace/recompile. Falls back to
    the stock path (handled by caller) on any failure.
    """
    global _runner
    if _runner is not None:
        return _runner
    import jax
    from jax.experimental.shard_map import shard_map
    from jax.sharding import Mesh, PartitionSpec

    from concourse import bass2jax, mybir as _mb

    bass2jax.install_neuronx_cc_hook()
    partition_name = (nc.partition_id_tensor.name
                      if nc.partition_id_tensor else None)
    in_names, out_names, out_avals = [], [], []
    for alloc in nc.m.functions[0].allocations:
        if not isinstance(alloc, _mb.MemoryLocationSet):
            continue
        name = alloc.memorylocations[0].name
        if alloc.kind == "ExternalInput":
            if name != partition_name:
                in_names.append(name)
        elif alloc.kind == "ExternalOutput":
            out_names.append(name)
            out_avals.append(jax.core.ShapedArray(
                tuple(alloc.tensor_shape), _mb.dt.np(alloc.dtype)))
    n_params = len(in_names)
    n_outs = len(out_avals)
    all_in_names = list(in_names) + list(out_names)
    if partition_name is not None:
        all_in_names.append(partition_name)

    def _body(*args):
        operands = list(args)
        if partition_name is not None:
            operands.append(bass2jax.partition_id_tensor())
        outs = bass2jax._bass_exec_p.bind(
            *operands,
            out_avals=tuple(out_avals),
            in_names=tuple(all_in_names),
            out_names=tuple(out_names),
            lowering_input_output_aliases=(),
            sim_require_finite=True,
            sim_require_nnan=True,
            nc=nc,
        )
        return tuple(outs)

    devices = jax.devices()[:NCORES]
    mesh = Mesh(np.asarray(devices), ("core",))
    specs = (PartitionSpec("core"),) * (n_params + n_outs)
    sharded = jax.jit(
        shard_map(_body, mesh=mesh, in_specs=specs,
                  out_specs=(PartitionSpec("core"),) * n_outs,
                  check_rep=False),
        donate_argnums=tuple(range(n_params, n_params + n_outs)),
        keep_unused=True,
    )
    _runner = (sharded, in_names, out_names, out_avals)
    return _runner


def _run_cached(nc, in_maps):
    sharded, in_names, out_names, out_avals = _get_runner(nc)
    concat_in = [
        np.concatenate([np.asarray(m[name]) for m in in_maps], axis=0)
        for name in in_names
    ]
    concat_zeros = [
        np.zeros((NCORES * a.shape[0], *a.shape[1:]), a.dtype)
        for a in out_avals
    ]
    out_arrs = sharded(*concat_in, *concat_zeros)
    return [
        {name: np.asarray(out_arrs[i]).reshape(NCORES, *out_avals[i].shape)[c]
         for i, name in enumerate(out_names)}
        for c in range(NCORES)
    ]


def kernel(x, W_route, A, Bw, lora_ind):
    global _nc_cache
    x = np.asarray(x, dtype=np.float32).reshape(NTOK, D)
    W_route = np.asarray(W_route, dtype=np.float32)
    A = np.asarray(A, dtype=np.float32)
    Bw = np.asarray(Bw, dtype=np.float32)
    lora_ind = np.asarray(lora_ind).astype(np.int64)

    # [D, 136] fp16: cols 0..127 are A rows in (g, e, r) order, 128.. W_route;
    # re-packed to [128, KD, FE] so each partition's DMA line is contiguous.
    A_all = A.transpose(1, 0, 2, 3).reshape(F, D)
    AWT = np.concatenate([A_all.T, W_route.T], axis=1).astype(np.float16)
    AWT_dev = np.ascontiguousarray(
        AWT.reshape(KD, 128, FE).transpose(1, 0, 2))
    # block-diagonal B^T with SCALING folded in: rows (g,e,r), cols (g,o)
    BTbd = (Bw.transpose(1, 0, 3, 2).reshape(G, E * R, OD)
            * SCALING).astype(np.float16)

    if _nc_cache is None:
        _nc_cache = _build()
    nc = _nc_cache

    with ThreadPoolExecutor(NCORES) as ex:
        xTs = list(ex.map(lambda c: _shard_xT(x, c), range(NCORES)))
    in_maps = [{"xT": xTs[c], "AWT": AWT_dev, "BT": BTbd}
               for c in range(NCORES)]

    try:
        results = _run_cached(nc, in_maps)
    except Exception:  # noqa: BLE001  (fall back to the stock SPMD path)
        global _runner
        _runner = None
        res = run_bass_kernel_spmd(nc, in_maps, core_ids=list(range(NCORES)),
                                   **_RUN_KWARGS)
        results = res.results
    _LAST["results"] = results

    compact = np.concatenate(
        [results[c]["out"] for c in range(NCORES)], axis=0)
    outp = np.zeros((NTOK, OUT), dtype=np.float32)
    outp[:, lora_ind] = compact.astype(np.float32)
    return outp.reshape(B, S, OUT)
